# revision 6
# baseline (speedup 1.0000x reference)
"""Trainium2 Bass kernel for nn_AutoregressiveBisectionInverter.

Math: the reference inverts f(x)_i = softplus(a_i)*x_i + (tanh(x) @ W^T)_i
per batch row via per-dimension bisection. W is strictly lower-triangular,
so f(x)_i is *linear* in x_i and the true inverse is the forward
substitution x_i = (y_i - sum_{j<i} W[i,j] tanh(x_j)) / softplus(a_i),
which the bisection approximates to |err| <= 1e-6.

On device we solve the equivalent fixed point
    x = D^{-1} (y - W tanh(x)),   D = diag(softplus(a))
with Jacobi sweeps. The iteration matrix is strictly lower triangular
(nilpotent), so the sweep is exact after <=64 iterations; numerically it
reaches the fp32 fixed point in ~11 sweeps. We run 16.

Per-core layout ([dim, batch] so that per-dim scaling is per-partition).
One fused input tile `init` [128, 128]:
    init[:, 0:64]  = lhsT_aug = [[ (diag(1/s) W)^T ], [ diag(-1/s) ]]
    init[:, 64:128] = rhs     = [[ t = tanh(x) (starts 0) ], [ y^T ]]
    acc [64, 64] PSUM = lhsT_aug.T @ rhs = diag(1/s)(W t - y) = -x_next
Each sweep is exactly two serial instructions:
    PE  : acc = lhsT_aug.T @ rhs
    ACT : rhs t-half = tanh(-acc)
Sharding: pure data parallel, 64 batch rows per core across 8 cores.
"""

import numpy as np

B, D = 512, 64
NCORES = 8
BLOC = B // NCORES  # 64 batch rows per core
NSWEEPS = 13

_CACHE = {}


def _build_nc():
    import concourse.bacc as bacc
    import concourse.tile as tile
    from concourse import mybir

    nc = bacc.Bacc("TRN2", target_bir_lowering=False)
    init = nc.dram_tensor("init", [2 * D, 2 * D], mybir.dt.float32, kind="ExternalInput")
    xT = nc.dram_tensor("xT", [D, BLOC], mybir.dt.float32, kind="ExternalOutput")

    with tile.TileContext(nc) as tc:
        with (
            tc.tile_pool(name="sb", bufs=1) as sb,
            tc.tile_pool(name="ps", bufs=1, space="PSUM") as ps,
        ):
            # Dummy early tanh so walrus's ACT_TABLE_LOAD for the tanh set
            # happens during the input DMA instead of delaying the first
            # real activation of the serial chain.
            warm = sb.tile([1, 1], mybir.dt.float32)
            nc.gpsimd.memset(warm[:], 0.0)
            nc.scalar.activation(warm[:], warm[:], mybir.ActivationFunctionType.Tanh)

            init_sb = sb.tile([2 * D, 2 * D], mybir.dt.float32)
            nc.sync.dma_start(init_sb[:], init[:])
            lhs_v = init_sb[:, 0:D]
            rhs_v = init_sb[:, D : 2 * D]

            acc = ps.tile([D, BLOC], mybir.dt.float32)

            # sweep 1 with t=0: acc = -diag(1/s) y
            nc.tensor.matmul(acc[:], lhs_v, rhs_v, start=True, stop=True)
            for _ in range(NSWEEPS - 1):
                # t = tanh(x) = tanh(-acc)
                nc.scalar.activation(
                    init_sb[0:D, D : 2 * D],
                    acc[:],
                    mybir.ActivationFunctionType.Tanh,
                    scale=-1.0,
                )
                nc.tensor.matmul(acc[:], lhs_v, rhs_v, start=True, stop=True)

            out_sb = sb.tile([D, BLOC], mybir.dt.float32)
            nc.scalar.mul(out_sb[:], acc[:], -1.0)  # x = -acc
            nc.sync.dma_start(xT[:], out_sb[:])

    nc.finalize()
    return nc


def kernel(y, a, W):
    from concourse.bass_utils import run_bass_kernel_spmd

    y = np.ascontiguousarray(np.asarray(y, dtype=np.float32))
    a = np.asarray(a, dtype=np.float32)
    W = np.asarray(W, dtype=np.float32)

    # Parameter-only host prep (O(D^2)): fold softplus scaling into the
    # static augmented stationary matrix.
    s = np.log1p(np.exp(a.astype(np.float64)))
    inv_s = (1.0 / s).astype(np.float32)
    w_scaled_T = (W * inv_s[:, None]).T  # [j, k] = W[k, j] / s_k

    base = np.zeros((2 * D, 2 * D), dtype=np.float32)
    base[0:D, 0:D] = w_scaled_T
    base[D : 2 * D, 0:D] = np.diag(-inv_s)

    if "nc" not in _CACHE:
        _CACHE["nc"] = _build_nc()
    nc = _CACHE["nc"]

    in_maps = []
    for c in range(NCORES):
        init_c = base.copy()
        init_c[D : 2 * D, D : 2 * D] = y[c * BLOC : (c + 1) * BLOC, :].T
        in_maps.append({"init": init_c})

    res = run_bass_kernel_spmd(nc, in_maps, list(range(NCORES)))
    out = np.empty((B, D), dtype=np.float32)
    for c in range(NCORES):
        out[c * BLOC : (c + 1) * BLOC, :] = res.results[c]["xT"].T
    return out


# revision 9
# speedup vs baseline: 1.0334x; 1.0334x over previous
"""Trainium2 Bass kernel for nn_AutoregressiveBisectionInverter.

Math: the reference inverts f(x)_i = softplus(a_i)*x_i + (tanh(x) @ W^T)_i
per batch row via per-dimension bisection. W is strictly lower-triangular,
so f(x)_i is *linear* in x_i and the true inverse is the forward
substitution x_i = (y_i - sum_{j<i} W[i,j] tanh(x_j)) / softplus(a_i),
which the bisection approximates to |err| <= 1e-6.

On device we solve the equivalent fixed point
    x = D^{-1} (y - W tanh(x)),   D = diag(softplus(a))
with Jacobi sweeps. The iteration matrix is strictly lower triangular
(nilpotent), so the sweep is exact after <=64 iterations; numerically it
reaches the fp32 fixed point in ~11 sweeps. We run 16.

Per-core layout ([dim, batch] so that per-dim scaling is per-partition).
One fused input tile `init` [128, 128]:
    init[:, 0:64]  = lhsT_aug = [[ (diag(1/s) W)^T ], [ diag(-1/s) ]]
    init[:, 64:128] = rhs     = [[ t = tanh(x) (starts 0) ], [ y^T ]]
    acc [64, 64] PSUM = lhsT_aug.T @ rhs = diag(1/s)(W t - y) = -x_next
Each sweep is exactly two serial instructions:
    PE  : acc = lhsT_aug.T @ rhs
    ACT : rhs t-half = tanh(-acc)
Sharding: pure data parallel, 64 batch rows per core across 8 cores.
"""

import numpy as np

B, D = 512, 64
NCORES = 8
BLOC = B // NCORES  # 64 batch rows per core
NSWEEPS = 12

_CACHE = {}


def _build_nc():
    import concourse.bacc as bacc
    import concourse.tile as tile
    from concourse import mybir

    nc = bacc.Bacc("TRN2", target_bir_lowering=False)
    # init layout [D, 3D]: cols 0:D = (diag(1/s) W)^T, D:2D = diag(-1/s),
    # 2D:3D = y^T slice. The t block of rhs is never DMA'd: sweep 1 uses
    # only the y half (K=64), and every later sweep reads t written by tanh.
    init = nc.dram_tensor("init", [D, 3 * D], mybir.dt.float32, kind="ExternalInput")
    xT = nc.dram_tensor("xT", [D, BLOC], mybir.dt.float32, kind="ExternalOutput")

    with tile.TileContext(nc) as tc:
        with (
            tc.tile_pool(name="sb", bufs=1) as sb,
            tc.tile_pool(name="ps", bufs=1, space="PSUM") as ps,
        ):
            # Dummy early tanh so walrus's ACT_TABLE_LOAD for the tanh set
            # happens during the input DMA instead of delaying the first
            # real activation of the serial chain.
            warm = sb.tile([1, 1], mybir.dt.float32)
            nc.gpsimd.memset(warm[:], 0.0)
            nc.scalar.activation(warm[:], warm[:], mybir.ActivationFunctionType.Tanh)

            init_sb = sb.tile([2 * D, 2 * D], mybir.dt.float32)
            # critical-path DMA: [diag | yT] into partitions 64:128
            nc.sync.dma_start(init_sb[D : 2 * D, :], init[:, D : 3 * D])
            # off-critical-path DMA: W''^T into partitions 0:64, cols 0:64
            nc.sync.dma_start(init_sb[0:D, 0:D], init[:, 0:D])
            lhs_v = init_sb[:, 0:D]
            rhs_v = init_sb[:, D : 2 * D]

            acc = ps.tile([D, BLOC], mybir.dt.float32)

            # sweep 1 with t=0: acc = -diag(1/s) y   (K=64, y half only)
            nc.tensor.matmul(
                acc[:],
                init_sb[D : 2 * D, 0:D],
                init_sb[D : 2 * D, D : 2 * D],
                start=True,
                stop=True,
            )
            for _ in range(NSWEEPS - 1):
                # t = tanh(x) = tanh(-acc)
                nc.scalar.activation(
                    init_sb[0:D, D : 2 * D],
                    acc[:],
                    mybir.ActivationFunctionType.Tanh,
                    scale=-1.0,
                )
                nc.tensor.matmul(acc[:], lhs_v, rhs_v, start=True, stop=True)

            out_sb = sb.tile([D, BLOC], mybir.dt.float32)
            nc.scalar.mul(out_sb[:], acc[:], -1.0)  # x = -acc
            nc.sync.dma_start(xT[:], out_sb[:])

    nc.finalize()
    return nc


def kernel(y, a, W):
    from concourse.bass_utils import run_bass_kernel_spmd

    y = np.ascontiguousarray(np.asarray(y, dtype=np.float32))
    a = np.asarray(a, dtype=np.float32)
    W = np.asarray(W, dtype=np.float32)

    # Parameter-only host prep (O(D^2)): fold softplus scaling into the
    # static augmented stationary matrix.
    s = np.log1p(np.exp(a.astype(np.float64)))
    inv_s = (1.0 / s).astype(np.float32)
    w_scaled_T = (W * inv_s[:, None]).T  # [j, k] = W[k, j] / s_k

    base = np.zeros((D, 3 * D), dtype=np.float32)
    base[:, 0:D] = w_scaled_T
    base[:, D : 2 * D] = np.diag(-inv_s)

    if "nc" not in _CACHE:
        _CACHE["nc"] = _build_nc()
    nc = _CACHE["nc"]

    in_maps = []
    for c in range(NCORES):
        init_c = base.copy()
        init_c[:, 2 * D : 3 * D] = y[c * BLOC : (c + 1) * BLOC, :].T
        in_maps.append({"init": init_c})

    res = run_bass_kernel_spmd(nc, in_maps, list(range(NCORES)))
    out = np.empty((B, D), dtype=np.float32)
    for c in range(NCORES):
        out[c * BLOC : (c + 1) * BLOC, :] = res.results[c]["xT"].T
    return out
